# revision 3
# baseline (speedup 1.0000x reference)
"""InfoNCE (CPIC) loss kernel v4 for Trainium2, 8 NeuronCores.

Math (B=1024, D=256):
  scores[i,j] = -0.5 * sum_d( log vc[j,d] + (y[i,d]-m[j,d])^2 / vc[j,d] )
    where vc = where(v < 1e-6, v + 1e-6, v)   (applied on host)
  mi_lower = log(B) + mean_i(diag_i - logsumexp_j scores[i,:])
  mi_upper = mean_i(diag_i - (logsumexp_{j!=i} scores[i,:] - log(B-1)))

Sharding (8 cores = 4 column groups x 2 row halves):
  core (g, h): columns j in [256g, +256), rows i in [512h, +512) as 4
  row tiles of 128.  The host rotates the row tiles so that when the
  core's j-range intersects its i-range the two diagonal 128-blocks
  are always local tiles 0 and 1, with the diag sub-block at cols
  [0:128) of tile 0 and [128:256) of tile 1 -- one uniform program
  for all cores; cores without diagonal get a zeroed +BIG mask.

Energy convention: psum holds E = -scores = 0.5*(dist + sum lv) so the
row-min reduce IS the exp bias (exp scale=-1), no negates anywhere.
  E = (y.y).T r' + y.T h + a''
    r' = 0.5/vc  (fast-recip + 0.5-scaled bf16 cast on the Act engine)
    h  = (-2m) . r'          = -m/vc      (packed bf16 tensor-tensor)
    q  = (-2m) . h           = 2 m^2/vc   (packed)
    w  = lvs + 0.5*qs        = sum_c(lv + m^2/vc)  (lv = ln vc)
    a''[j] = 0.5 * sum_d w   via a halfones(0.5)-weight matmul that
    broadcast-adds into every row.  The host passes mT pre-scaled by
    -2 so every elementwise op is a packed bf16 mult (2 elem/cyc).
  bf16 end-to-end rel err ~1e-4 vs the 2e-2 tolerance.
Per tile: diag extract (mask-mul + row-sum), +BIG at diag (PE),
bias = min_j E, e = exp(-E + bias) with fused row-sum on Act.
DMAs are split per contraction chunk across three queues so the
first matmul starts one transfer earlier.
Device output per core [128, 10] f32: E_diag0, E_diag1, min0-3 (=
-rowmax of scores), S0-3; host merges the 4 column groups per row
(logaddexp) and takes the means.
"""

import sys

import numpy as np
import ml_dtypes

sys.path.insert(0, "/opt/trn_rl_repo")

import concourse.bass as bass  # noqa: E402,F401
import concourse.bacc as bacc  # noqa: E402
import concourse.tile as tile  # noqa: E402
from concourse.tile import add_dep_helper  # noqa: E402
import concourse.hw_specs as hw_specs  # noqa: E402
from concourse import mybir  # noqa: E402
from concourse import bass_utils  # noqa: E402
from concourse.dve_ops import (  # noqa: E402
    RECIP_APPROX_FAST_CONSTS,
    RECIPROCAL_APPROX_FAST,
)
from contextlib import ExitStack  # noqa: E402

B = 1024
D = 256
NCORES = 8
NG = 4          # column groups
NH = 2          # row halves
JC = B // NG    # 256 cols per core
IC = B // NH    # 512 rows per core
NT = IC // 128  # 4 row tiles per core
KC = D // 128   # 2 contraction chunks
THRESHOLD = 1e-6
BIG = float(2.0**60)

F32 = mybir.dt.float32
BF16 = mybir.dt.bfloat16
AX = mybir.AxisListType
OP = mybir.AluOpType
AF = mybir.ActivationFunctionType

_ACT_SET = "natural_log_exp_and_others"


def _patch_act_tables():
    """Make every activation resolve to the one set that holds ln+exp+
    square+copy, so only one ACT_TABLE_LOAD (~1.3us) is emitted."""
    if getattr(hw_specs, "_ant_act_patch", None):
        return
    orig = hw_specs.get_activation_tables

    def patched(arch):
        tabs = orig(arch)
        if _ACT_SET not in tabs:
            return tabs
        return {k: (v if k == _ACT_SET else set()) for k, v in tabs.items()}

    hw_specs._ant_act_patch = True
    hw_specs.get_activation_tables = patched
    for mod in (bacc, bass):
        if hasattr(mod, "get_activation_tables"):
            mod.get_activation_tables = patched


def _build():
    _patch_act_tables()
    nc = bacc.Bacc("TRN2", target_bir_lowering=False, debug=False, num_devices=8)
    yT = nc.declare_dram_parameter("yT", [D, IC], BF16, isOutput=False)
    vT = nc.declare_dram_parameter("vT", [D, JC], F32, isOutput=False)
    mT = nc.declare_dram_parameter("mT", [D, JC], BF16, isOutput=False)  # -2m
    # consts: [iden | bmsk(+BIG*flag*I) | halfones(0.5)]
    consts = nc.declare_dram_parameter("consts", [128, 384], BF16, isOutput=False)
    constsf = nc.declare_dram_parameter("constsf", [128, 128], F32, isOutput=False)
    out = nc.declare_dram_parameter("out", [128, 10], F32, isOutput=True)

    with ExitStack() as ctx:
        tc = ctx.enter_context(tile.TileContext(nc))
        pool = ctx.enter_context(tc.tile_pool(name="main", bufs=1))
        ppool = ctx.enter_context(tc.tile_pool(name="psum", bufs=1, space="PSUM"))

        y_t = pool.tile([128, KC * IC], BF16, name="y")       # [128, 2*512]
        y2_t = pool.tile([128, KC * IC], BF16, name="y2")
        v_t = pool.tile([128, KC * JC], F32, name="v")        # [128, 2*256]
        m_t = pool.tile([128, KC * JC], BF16, name="m")       # -2m
        rf_t = pool.tile([128, KC * JC], F32, name="rf")
        r_t = pool.tile([128, KC * JC], BF16, name="r")       # 0.5/vc
        h_t = pool.tile([128, KC * JC], BF16, name="h")
        q_t = pool.tile([128, KC * JC], BF16, name="q")
        lv_t = pool.tile([128, KC * JC], BF16, name="lv")
        lvs_t = pool.tile([128, JC], BF16, name="lvs")
        qs_t = pool.tile([128, JC], BF16, name="qs")
        w_t = pool.tile([128, JC], BF16, name="w")
        consts_t = pool.tile([128, 384], BF16, name="consts")
        constsf_t = pool.tile([128, 128], F32, name="constsf")
        iden_t = consts_t[:, 0:128]
        bmsk_t = consts_t[:, 128:256]
        hones_t = consts_t[:, 256:384]
        e_t = pool.tile([128, NT * JC], BF16, name="e")
        scrd_t = pool.tile([128, 256], F32, name="scrd")
        o_t = pool.tile([128, 10], F32, name="o")

        psum = [ppool.tile([128, JC], F32, name=f"s{t}") for t in range(NT)]

        yT3 = yT.rearrange("(c p) i -> p c i", p=128)
        vT3 = vT.rearrange("(c p) j -> p c j", p=128)
        mT3 = mT.rearrange("(c p) j -> p c j", p=128)
        y3 = y_t[:].rearrange("p (c i) -> p c i", c=KC)
        v3 = v_t[:].rearrange("p (c j) -> p c j", c=KC)
        m3 = m_t[:].rearrange("p (c j) -> p c j", c=KC)

        # three DMA queues, both chunks of each chain back-to-back
        nc.sync.dma_start(out=v3[:, 0, :], in_=vT3[:, 0, :])
        nc.scalar.dma_start(out=y3[:, 0, :], in_=yT3[:, 0, :])
        nc.gpsimd.dma_start(out=m3, in_=mT3)
        nc.sync.dma_start(out=v3[:, 1, :], in_=vT3[:, 1, :])
        nc.scalar.dma_start(out=y3[:, 1, :], in_=yT3[:, 1, :])
        nc.gpsimd.dma_start(out=consts_t[:], in_=consts[:, :])
        nc.gpsimd.dma_start(out=constsf_t[:], in_=constsf[:, :])

        c = RECIP_APPROX_FAST_CONSTS
        js = [slice(cc * JC, (cc + 1) * JC) for cc in range(KC)]
        ws = [slice(cc * IC, (cc + 1) * IC) for cc in range(KC)]
        with nc.allow_low_precision(reason="bf16 operands"):
            casts = []
            for cc in range(KC):
                nc.vector._custom_dve(
                    RECIPROCAL_APPROX_FAST, out=rf_t[:, js[cc]],
                    in0=v_t[:, js[cc]],
                    s0=c["s0"], s1=c["s1"], imm2=c["imm2"],
                )
                nc.vector.tensor_mul(y2_t[:, ws[cc]], y_t[:, ws[cc]],
                                     y_t[:, ws[cc]])
                casts.append(nc.scalar.activation(r_t[:, js[cc]], rf_t[:, js[cc]],
                                                  AF.Copy, scale=0.5))
                nc.vector.tensor_mul(h_t[:, js[cc]], m_t[:, js[cc]],
                                     r_t[:, js[cc]])
            nc.vector.tensor_mul(q_t[:], m_t[:], h_t[:])
            i_ln = nc.scalar.activation(lv_t[:], v_t[:], AF.Ln)
            # keep the Act program cast-first: the r casts gate the first
            # matmuls; Ln's consumer (lvs) is needed much later
            add_dep_helper(i_ln.ins, casts[-1].ins, sync=False,
                           reason="act order")
            nc.vector.tensor_add(qs_t[:], q_t[:, js[0]], q_t[:, js[1]])
            nc.vector.tensor_add(lvs_t[:], lv_t[:, js[0]], lv_t[:, js[1]])
            nc.vector.scalar_tensor_tensor(
                out=w_t[:], in0=qs_t[:], scalar=0.5, in1=lvs_t[:],
                op0=OP.mult, op1=OP.add,
            )

        # data matmuls, chunk-major then term-major: the four y2.r
        # matmuls of a chunk run while h of that chunk is still being
        # computed on the DVE
        TORD = [2, 3, 0, 1]
        started = set()
        for cc in range(KC):
            for t in TORD:
                isl = slice(cc * IC + t * 128, cc * IC + (t + 1) * 128)
                nc.tensor.matmul(
                    psum[t][:], y2_t[:, isl], r_t[:, js[cc]],
                    start=(t not in started), stop=False,
                )
                started.add(t)
            for t in TORD:
                isl = slice(cc * IC + t * 128, cc * IC + (t + 1) * 128)
                nc.tensor.matmul(
                    psum[t][:], y_t[:, isl], h_t[:, js[cc]],
                    start=False, stop=False,
                )
        # a''[j] broadcast-add closes each tile's accumulation group
        for t in TORD:
            nc.tensor.matmul(
                psum[t][:], hones_t, w_t[:],
                start=False, stop=True,
            )

        dsl = [slice(0, 128), slice(128, 256)]
        # DVE tail: mins for the no-diag tiles first, then diag extracts;
        # each +BIG matmul waits on its extract's mask-mul (WAR)
        nc.vector.tensor_reduce(
            out=o_t[:, 4:5], in_=psum[2][:], axis=AX.X, op=OP.min)
        nc.vector.tensor_reduce(
            out=o_t[:, 5:6], in_=psum[3][:], axis=AX.X, op=OP.min)
        for t in range(2):
            nc.vector.tensor_mul(scrd_t[:, dsl[t]], psum[t][:, dsl[t]],
                                 constsf_t[:])
            nc.vector.tensor_reduce(
                out=o_t[:, t:t + 1], in_=scrd_t[:, dsl[t]],
                axis=AX.X, op=OP.add,
            )
        for t in range(2):
            nc.tensor.matmul(
                psum[t][:, dsl[t]], iden_t, bmsk_t,
                start=False, stop=True, skip_group_check=True,
            )
        for t in range(2):
            nc.vector.tensor_reduce(
                out=o_t[:, 2 + t:3 + t], in_=psum[t][:], axis=AX.X, op=OP.min)

        # e = exp(-E + bias), S = fused row sum
        for t in TORD:
            with nc.allow_low_precision(reason="e only feeds f32 accum"):
                nc.scalar.activation(
                    e_t[:, t * JC:(t + 1) * JC], psum[t][:], AF.Exp,
                    bias=o_t[:, 2 + t:3 + t], scale=-1.0,
                    accum_out=o_t[:, 6 + t:7 + t],
                )

        nc.sync.dma_start(out=out[:, :], in_=o_t[:])

    nc.finalize()
    return nc


_CACHE = {}


def _get_nc():
    if "nc" not in _CACHE:
        _CACHE["nc"] = _build()
    return _CACHE["nc"]


BF = ml_dtypes.bfloat16


def _tile_order(g, h):
    """Global 128-row-block indices for core (g,h), rotated so the two
    blocks matching the core's j-range come first (when present)."""
    rho = (2 * g) % NT
    return [4 * h + ((rho + t) % NT) for t in range(NT)]


def _in_maps(x_mean, x_vars, y):
    m = np.asarray(x_mean, dtype=np.float32)
    v = np.asarray(x_vars, dtype=np.float32)
    yv = np.asarray(y, dtype=np.float32)
    vc = np.where(v < np.float32(THRESHOLD), v + np.float32(THRESHOLD), v)

    mT = np.ascontiguousarray((-2.0 * m).T.astype(BF))   # [D, B]
    vT = np.ascontiguousarray(vc.T.astype(np.float32))
    yT = np.ascontiguousarray(yv.T.astype(BF))

    iden = np.eye(128, dtype=np.float32)
    base = np.zeros((128, 384), np.float32)
    base[:, 0:128] = iden
    base[:, 256:384] = 0.5

    maps = []
    for core in range(NCORES):
        g, h = core % NG, core // NG
        order = _tile_order(g, h)
        yTs = np.ascontiguousarray(
            np.concatenate([yT[:, 128 * b:128 * (b + 1)] for b in order], axis=1)
        )
        consts = base.copy()
        if g // 2 == h:
            consts[:, 128:256] = iden * np.float32(BIG)
        maps.append({
            "constsf": iden,
            "yT": yTs.astype(BF),
            "vT": np.ascontiguousarray(vT[:, JC * g:JC * (g + 1)]),
            "mT": np.ascontiguousarray(mT[:, JC * g:JC * (g + 1)]),
            "consts": consts.astype(BF),
        })
    return maps


def _combine(results):
    # per core [128, 10]: E_diag0, E_diag1, min0-3 (= -rowmax scores), S0-3
    lse_nd = np.full(B, -np.inf)           # logsumexp excluding diag, per row
    diag = np.zeros(B)
    for core in range(NCORES):
        g, h = core % NG, core // NG
        o = results[core]["out"].astype(np.float64)
        order = _tile_order(g, h)
        for t, blk in enumerate(order):
            rows = slice(128 * blk, 128 * (blk + 1))
            L = -o[:, 2 + t] + np.log(o[:, 6 + t])
            lse_nd[rows] = np.logaddexp(lse_nd[rows], L)
            if t < 2 and g // 2 == h:
                diag[rows] = -o[:, t]
    lse_f = np.logaddexp(lse_nd, diag)
    mi_lower = np.log(float(B)) + np.mean(diag - lse_f)
    mi_upper = np.mean(diag - lse_nd) + np.log(float(B - 1))
    return np.array([mi_lower, mi_upper], dtype=np.float32)


def _run(x_mean, x_vars, y, **kw):
    nc = _get_nc()
    res = bass_utils.run_bass_kernel_spmd(
        nc, _in_maps(x_mean, x_vars, y), list(range(NCORES)), **kw
    )
    return _combine(res.results), res


def kernel(x_mean, x_vars, y):
    return _run(x_mean, x_vars, y)[0]
